# revision 73
# baseline (speedup 1.0000x reference)
"""Multi-head causal attention (B=2, S=2048, D=1024, H=16, HD=64) on 8 TRN2 cores.

Sharding: data + tensor parallel. Core c handles batch b = c // 4 and head
group g = c % 4 (4 heads = 256 of the 1024 hidden dims). Wq/Wk/Wv are split
column-wise, Wo row-wise; each core computes a partial [D, S] output (its
heads' contribution, transposed), and the host sums the 4 partials per batch.

On-device layout (per core): everything is computed "transposed" so the PE
contraction dim always sits on partitions:
  xT [D, S] -> Q2T/K2T [128 (2 heads x 64 dims), S] -> scoresT [k, q]
  -> exp -> PV with a ones-column appended to V (denominator lands on
  partition 64) -> normalize -> O^T [256, S] -> Wo^T partial [D, S].
All matmuls run as float32r (full PE rate at free-dim >=256, ~1e-4 rel err).

Causal handling: for a q-chunk of 512, k-tiles strictly below the diagonal
are computed full-width; the 4 k-tiles overlapping the diagonal are computed
only on their live column range [w:512] (w = 128 * tile-offset), with one
resident [128,128] triangle mask added to the diagonal block. Columns left
of w are never computed, masked, exp'd, or fed to PV. The kernel runs as a
pipeline over S-stripes (load stripe -> V -> Q/K proj -> attention chunk ->
deferred output projection), with stripe 0 additionally fed in an
s-tile-major layout so the first matmul starts after 0.5 MB of DMA.
"""

import sys

sys.path.insert(0, "/opt/trn_rl_repo")

import numpy as np
import ml_dtypes

import concourse.bass as bass
import concourse.tile as tile
from concourse import bacc, mybir
from concourse.bass_utils import run_bass_kernel_spmd

B, S, D, H, HD = 2, 2048, 1024, 16, 64
NCORES = 8
HPC = H // (NCORES // B)          # heads per core = 4
GD = HPC * HD                     # head-group width = 256
CH = 512                          # q-chunk (max fp32 moving free dim)
NCH = S // CH                     # 4 q-chunks
KT = S // 128                     # 16 k-tiles
ND = D // 128                     # 8 d-tiles
NEG = -30000.0                    # mask value; exp(NEG/8) == 0 in fp32

f32 = mybir.dt.float32
f32r = mybir.dt.float32r
bf16 = mybir.dt.bfloat16

_prog_cache = {}


def _build(variant):
    """variant: 'causal' (triangle mask resident, diagonal narrowing),
    'full' (no masking), 'masked' (arbitrary mask streamed from DRAM)."""
    nc = bacc.Bacc("TRN2", target_bir_lowering=False, debug=False,
                   num_devices=NCORES)

    xt_ext = nc.declare_dram_parameter("xt", [128, NCH, ND, CH], bf16,
                                       isOutput=False)
    xt0_ext = nc.declare_dram_parameter("xt0", [128, 4, ND, 128], bf16,
                                        isOutput=False)
    wq_ext = nc.declare_dram_parameter("wq4", [128, ND, GD], bf16,
                                       isOutput=False)
    wk_ext = nc.declare_dram_parameter("wk4", [128, ND, GD], bf16,
                                       isOutput=False)
    wv_ext = nc.declare_dram_parameter("wv4", [128, ND, GD], bf16,
                                       isOutput=False)
    wo_ext = nc.declare_dram_parameter("wo4", [128, 2, D], bf16,
                                       isOutput=False)
    bq_ext = nc.declare_dram_parameter("bq4", [GD], f32, isOutput=False)
    bk_ext = nc.declare_dram_parameter("bk4", [GD], f32, isOutput=False)
    bv_ext = nc.declare_dram_parameter("bv4", [GD], f32, isOutput=False)
    bo_ext = nc.declare_dram_parameter("bo1", [D], f32, isOutput=False)
    if variant == "causal":
        mk_ext = nc.declare_dram_parameter("tri", [128, 128], bf16,
                                           isOutput=False)
    elif variant == "masked":
        mk_ext = nc.declare_dram_parameter("mkf", [KT, NCH, 128, CH], bf16,
                                           isOutput=False)
    out_ext = nc.declare_dram_parameter("out", [128, NCH, 2, ND // 2, CH],
                                        bf16, isOutput=True)

    Ident = mybir.ActivationFunctionType.Identity
    Exp = mybir.ActivationFunctionType.Exp

    with tile.TileContext(nc) as tc:
        with tc.tile_pool(name="consts", bufs=1) as consts, \
             tc.tile_pool(name="qk", bufs=2) as qk_pool, \
             tc.tile_pool(name="ptp", bufs=(8 if variant == "causal" else 5)) as pt_pool, \
             tc.tile_pool(name="scr", bufs=2) as sc_pool, \
             tc.tile_pool(name="opp", bufs=10) as op_pool, \
             tc.tile_pool(name="outp", bufs=2) as outp, \
             tc.tile_pool(name="pp", bufs=2, space="PSUM") as pp, \
             tc.tile_pool(name="sp", bufs=2, space="PSUM") as sp, \
             tc.tile_pool(name="vp", bufs=2, space="PSUM") as vp:

            # ---- PE warm-up: absorb the p-state ramp while DMAs land ----
            warm_sb = consts.tile([128, CH], bf16)
            nc.vector.memset(warm_sb, 0.0)
            for i in range(12):
                wp = pp.tile([128, CH], f32, tag="pp", name=f"wp{i}")
                nc.tensor.matmul(wp[0:64, :], warm_sb[:, 0:64], warm_sb,
                                 start=True, stop=True)

            # ---- resident loads (one sync queue, in need order) ----
            wv_sb = consts.tile([128, ND, GD], bf16)
            xts0 = consts.tile([128, 4, ND, 128], bf16)
            xts = [None] + [consts.tile([128, ND, CH], bf16, name=f"xts{i}")
                            for i in range(1, NCH)]
            for tl in range(2):
                nc.sync.dma_start(out=xts0[:, tl], in_=xt0_ext[:, tl])
            nc.sync.dma_start(out=wv_sb[:, 0:4], in_=wv_ext[:, 0:4, :])
            nc.sync.dma_start(out=wv_sb[:, 4:8], in_=wv_ext[:, 4:8, :])
            for tl in range(2, 4):
                nc.sync.dma_start(out=xts0[:, tl], in_=xt0_ext[:, tl])
            wq_sb = consts.tile([128, ND, GD], bf16)
            wk_sb = consts.tile([128, ND, GD], bf16)
            bv_row = consts.tile([1, GD], f32)
            nc.gpsimd.dma_start(out=bv_row, in_=bv_ext[None, :])
            if variant == "causal":
                tri_sb = consts.tile([128, 128], bf16)
                nc.gpsimd.dma_start(out=tri_sb, in_=mk_ext[:, :])
            bq_sb = consts.tile([128, 2], f32)
            bk_sb = consts.tile([128, 2], f32)
            nc.gpsimd.dma_start(out=bq_sb, in_=bq_ext.rearrange("(t p) -> p t", p=128))
            nc.gpsimd.dma_start(out=bk_sb, in_=bk_ext.rearrange("(t p) -> p t", p=128))
            bo_sb = consts.tile([128, ND], f32)
            nc.gpsimd.dma_start(out=bo_sb, in_=bo_ext.rearrange("(t p) -> p t", p=128))
            wo_sb = consts.tile([128, 2, D], bf16)
            nc.gpsimd.dma_start(out=wo_sb, in_=wo_ext[:, :, :])
            bvb = consts.tile([128, GD], f32)
            nc.gpsimd.partition_broadcast(bvb[:, :], bv_row[:, :])
            ones_c = consts.tile([128, KT, HPC, 1], bf16)
            nc.vector.memset(ones_c, 1.0)
            actwarm = consts.tile([1, 1], f32)
            nc.scalar.activation(out=actwarm, in_=bvb[0:1, 0:1],
                                 func=Exp, scale=1.0)

            vau = consts.tile([128, KT, HPC, HD + 1], bf16)
            ot_sb = consts.tile([128, 2, S], bf16)

            # ones-column of V_aug (PV denominator trick), single strided copy
            nc.vector.tensor_copy(out=vau[:, :, :, HD:HD + 1], in_=ones_c)

            # ---- stripe-major main loop: for each 512-col stripe of S:
            #      load xt stripe -> V s-tiles -> QK projections (both pairs)
            #      -> attention chunk c (all 4 heads) -> output projection ----
            q2ts, k2ts = [], []
            for p in range(2):
                q2t_p = qk_pool.tile([128, S], bf16, tag="q2t", name=f"q2t{p}")
                k2t_p = qk_pool.tile([128, S], bf16, tag="k2t", name=f"k2t{p}")
                q2ts.append(q2t_p)
                k2ts.append(k2t_p)

            def final_proj(c, dhs=(0, 1), qr=(0, CH), half=None):
                # output projection for one chunk (deferred by one stripe)
                q0, q1 = qr
                for dh in dhs:
                    o_big = outp.tile([128, ND // 2, CH], bf16, tag="out")
                    ds = range(dh * (ND // 2), (dh + 1) * (ND // 2))
                    if half is not None:
                        ds = ds[half * 2:half * 2 + 2]
                    for d in ds:
                        f_ps = pp.tile([128, CH], f32, tag="pp")
                        for t in range(2):
                            nc.tensor.matmul(
                                f_ps[:, q0:q1],
                                wo_sb[:, t, d * 128:(d + 1) * 128],
                                ot_sb[:, t, c * CH + q0:c * CH + q1],
                                start=(t == 0), stop=(t == 1))
                        dd = d - dh * (ND // 2)
                        nc.vector.tensor_scalar_add(
                            out=o_big[:, dd, q0:q1], in0=f_ps[:, q0:q1],
                            scalar1=bo_sb[:, d:d + 1])
                        if dd % 2 == 1:
                            nc.sync.dma_start(
                                out=out_ext[:, c, dh, dd - 1:dd + 1, q0:q1],
                                in_=o_big[:, dd - 1:dd + 1, q0:q1])


            from collections import deque
            deferred = deque()
            inj = []
            acc = [0.0]
            ratio = [0.0]

            def attn_chunk(c, inject_list=None):
                # attention chunk c; PV accumulated TRANSPOSED (out[q, 65],
                # ones-column denominator in col 64; one bank-wide PSUM
                # group per head). Scores+exp run ahead of the deferred
                # PV consumers; each q-subtile normalizes and transposes
                # back as soon as its diagonal tile retires. Filler work
                # (next stripe's projections, previous chunk's output
                # projection) is paced into the stream.
                # force-drain filler that must precede this chunk (its
                # own projections), then add the new filler to the shared
                # paced queue
                while inj:
                    inj.pop(0)()
                inj.extend(inject_list or [])
                nunits = 4 * (2 * c + 4) if variant == "causal" else 4 * KT
                ratio[0] = len(inj) / (nunits * 1.4)
                acc[0] = 0.0

                def pump(lag):
                    while len(deferred) > lag:
                        deferred.popleft()()
                    acc[0] += ratio[0]
                    while acc[0] >= 1.0 and inj:
                        inj.pop(0)()
                        acc[0] -= 1.0

                for p in range(2):
                    q2t, k2t = q2ts[p], k2ts[p]
                    opairs = [op_pool.tile([128, 128], bf16, tag="op",
                                           name=f"op{c}_{p}_{j}")
                              for j in range(4)]
                    for hp in range(2):
                        h = 2 * p + hp
                        lo, hi = hp * 64, hp * 64 + 64
                        qs = q2t[lo:hi, c * CH:(c + 1) * CH]
                        pvt = vp.tile([128, 4, HD + 1], f32, tag="pv")

                        def do_pv(t, ptl_ap, j0, last_t, pvt=pvt, h=h):
                            for j in range(j0, 4):
                                nc.tensor.matmul(
                                    pvt[:, j, :],
                                    ptl_ap[:, j * 128:(j + 1) * 128],
                                    vau[:, t, h, :],
                                    start=(t == 0 and j == j0),
                                    stop=(last_t and j == 3),
                                    skip_group_check=True)

                        def norm_j(j, pvt=pvt, lo=lo, hi=hi, opairs=opairs,
                                   hp=hp, p=p, c=c, last=(hp == 1 and p == 1
                                                          and c == NCH - 1)):
                            rcp = sc_pool.tile([128, 1, 1], f32, tag="rc")
                            nc.vector.reciprocal(rcp,
                                                 pvt[:, j:j + 1, HD:HD + 1])
                            nc.vector.tensor_scalar_mul(
                                out=opairs[j][:, lo:hi],
                                in0=pvt[:, j, 0:HD],
                                scalar1=rcp[:, 0, :])
                            if hp == 1:
                                nc.sync.dma_start_transpose(
                                    out=ot_sb[:, p, c * CH + j * 128:
                                              c * CH + (j + 1) * 128],
                                    in_=opairs[j])


                        if variant == "causal":
                            nfull = 4 * c
                            for tp in range(nfull // 2):
                                t0 = 2 * tp
                                s2 = sp.tile([128, 2, CH], f32, tag="sc")
                                for k in range(2):
                                    nc.tensor.matmul(
                                        s2[:, k, :],
                                        k2t[lo:hi, (t0 + k) * 128:
                                            (t0 + k + 1) * 128],
                                        qs, start=True, stop=True)
                                ptl = pt_pool.tile([128, 2, CH], bf16,
                                                   tag="pt")
                                nc.scalar.activation(out=ptl, in_=s2,
                                                     func=Exp, scale=0.125)
                                deferred.append(
                                    lambda t0=t0, ptl=ptl, f=do_pv: (
                                        f(t0, ptl[:, 0, :], 0, False),
                                        f(t0 + 1, ptl[:, 1, :], 0, False)))
                                pump(2)
                            for j in range(4):      # diagonal band
                                t = 4 * c + j
                                w = 128 * j
                                s2 = sp.tile([128, 2, CH], f32, tag="sc")
                                s_ps = s2[:, 0, :]
                                nc.tensor.matmul(
                                    s_ps[:, w:CH],
                                    k2t[lo:hi, t * 128:(t + 1) * 128],
                                    q2t[lo:hi, c * CH + w:(c + 1) * CH],
                                    start=True, stop=True)
                                nc.vector.tensor_add(s_ps[:, w:w + 128],
                                                     s_ps[:, w:w + 128],
                                                     tri_sb)
                                ptl = pt_pool.tile([128, 2, CH], bf16,
                                                   tag="pt")
                                nc.scalar.activation(out=ptl[:, 0, w:CH],
                                                     in_=s_ps[:, w:CH],
                                                     func=Exp, scale=0.125)
                                deferred.append(
                                    lambda t=t, ptl=ptl, j=j, f=do_pv,
                                    g=norm_j: (f(t, ptl[:, 0, :], j, j == 3),
                                               g(j)))
                                pump(2)
                        else:
                            for t in range(KT):
                                s2 = sp.tile([128, 2, CH], f32, tag="sc")
                                s_ps = s2[:, 0, :]
                                nc.tensor.matmul(
                                    s_ps,
                                    k2t[lo:hi, t * 128:(t + 1) * 128],
                                    qs, start=True, stop=True)
                                if variant == "masked":
                                    mt = pt_pool.tile([128, CH], bf16,
                                                      tag="mkt")
                                    nc.sync.dma_start(
                                        out=mt, in_=mk_ext[t, c])
                                    nc.vector.tensor_add(s_ps, s_ps, mt)
                                ptl = pt_pool.tile([128, 2, CH], bf16,
                                                   tag="pt")
                                nc.scalar.activation(out=ptl[:, 0, :],
                                                     in_=s_ps,
                                                     func=Exp, scale=0.125)
                                if t < KT - 1:
                                    deferred.append(
                                        lambda t=t, ptl=ptl, f=do_pv:
                                        f(t, ptl[:, 0, :], 0, False))
                                else:
                                    deferred.append(
                                        lambda t=t, ptl=ptl, f=do_pv,
                                        g=norm_j:
                                        (f(t, ptl[:, 0, :], 0, True),
                                         g(0), g(1), g(2), g(3)))
                                pump(2)

            def v_group(t, c):
                v4 = pp.tile([128, CH], f32, tag="pp")
                for d in range(ND):
                    tl = t - 4 * c
                    xl = (xts0[:, tl, d, :] if c == 0 else
                          xts[c][:, d, tl * 128:(tl + 1) * 128])
                    nc.tensor.matmul(
                        v4[:, :GD], xl, wv_sb[:, d, :],
                        start=(d == 0), stop=(d == ND - 1))
                nc.vector.tensor_add(
                    vau[:, t, :, 0:HD],
                    v4[:, 0:GD].rearrange("p (h e) -> p h e", h=HPC),
                    bvb.rearrange("p (h e) -> p h e", h=HPC))

            def qk_group(c, p, w_sb, b_sb, dst):
                pr = pp.tile([128, CH], f32, tag="pp")
                for d in range(ND):
                    xr = (xts0[:, :, d, :] if c == 0 else xts[c][:, d, :])
                    nc.tensor.matmul(
                        pr, w_sb[:, d, p * 128:(p + 1) * 128],
                        xr, start=(d == 0), stop=(d == ND - 1))
                nc.vector.tensor_scalar_add(
                    out=dst[:, c * CH:(c + 1) * CH], in0=pr,
                    scalar1=b_sb[:, p:p + 1])

            def proj_groups(c):
                gs = [lambda t=t, c=c: v_group(t, c)
                      for t in range(4 * c, 4 * c + 4)]
                for p in range(2):
                    gs.append(lambda c=c, p=p: qk_group(
                        c, p, wq_sb, bq_sb, q2ts[p]))
                    gs.append(lambda c=c, p=p: qk_group(
                        c, p, wk_sb, bk_sb, k2ts[p]))
                return gs

            if variant == "causal":
                # chunk c's attention stream carries stripe c+1's
                # projections and chunk c-1's output projection as paced
                # PE filler
                for c in range(NCH):
                    if c == 0:
                        nc.sync.dma_start(out=wq_sb, in_=wq_ext[:, :, :])
                        nc.sync.dma_start(out=wk_sb, in_=wk_ext[:, :, :])
                        nc.sync.dma_start(out=xts[1], in_=xt_ext[:, 1])
                        for g in proj_groups(0):
                            g()
                    filler = []
                    if c == 1:
                        filler.append(lambda: nc.sync.dma_start(
                            out=xts[2], in_=xt_ext[:, 2]))
                    elif c == 2:
                        filler.append(lambda: nc.sync.dma_start(
                            out=xts[3], in_=xt_ext[:, 3]))
                    if c > 0:
                        filler.append(lambda c=c: final_proj(c - 1, (0,)))
                        filler.append(lambda c=c: final_proj(c - 1, (1,)))
                    if c + 1 < NCH:
                        filler += proj_groups(c + 1)
                    attn_chunk(c, inject_list=filler)
                while len(deferred) > 0:
                    deferred.popleft()()
                for g in inj:
                    g()
                final_proj(NCH - 1)
            else:
                for c in range(NCH):
                    for t in range(4 * c, 4 * c + 4):
                        v_group(t, c)
                    if c == 0:
                        nc.sync.dma_start(out=wq_sb, in_=wq_ext[:, :, :])
                        nc.sync.dma_start(out=wk_sb, in_=wk_ext[:, :, :])
                        nc.sync.dma_start(out=xts[1], in_=xt_ext[:, 1])
                    elif c == 1:
                        nc.sync.dma_start(out=xts[2], in_=xt_ext[:, 2])
                    elif c == 2:
                        nc.sync.dma_start(out=xts[3], in_=xt_ext[:, 3])
                    for p in range(2):
                        qk_group(c, p, wq_sb, bq_sb, q2ts[p])
                        qk_group(c, p, wk_sb, bk_sb, k2ts[p])
                    attn_chunk(c)
                    while len(deferred) > 0:
                        deferred.popleft()()
                    final_proj(c)


    nc.compile()
    return nc


def _get_prog(variant):
    if variant not in _prog_cache:
        _prog_cache[variant] = _build(variant)
    return _prog_cache[variant]


def _classify_mask(mask):
    m = np.asarray(mask).reshape(S, S).astype(bool)
    tril = np.tril(np.ones((S, S), bool))
    if (m == tril).all():
        return "causal", None
    if m.all():
        return "full", None
    return "masked", m


def _tri_mask():
    # diagonal-block triangle in scoresT layout: 0 if kk <= qq else NEG
    kk = np.arange(128)[:, None]
    qq = np.arange(128)[None, :]
    return np.where(kk <= qq, 0.0, NEG).astype(ml_dtypes.bfloat16)


def _full_masks(m):
    # mkf[t, c, kk, qq] = 0 if m[c*CH+qq, t*128+kk] else NEG  (scoresT layout)
    mt = np.where(m.T, 0.0, NEG).astype(ml_dtypes.bfloat16)  # [k, q]
    return np.ascontiguousarray(
        mt.reshape(KT, 128, NCH, CH).transpose(0, 2, 1, 3))


def kernel(x, mask, wq, bq, wk, bk, wv, bv, wo, bo):
    x = np.asarray(x, dtype=np.float32)
    wq = np.asarray(wq, dtype=np.float32)
    wk = np.asarray(wk, dtype=np.float32)
    wv = np.asarray(wv, dtype=np.float32)
    wo = np.asarray(wo, dtype=np.float32)
    bq = np.asarray(bq, dtype=np.float32)
    bk = np.asarray(bk, dtype=np.float32)
    bv = np.asarray(bv, dtype=np.float32)
    bo = np.asarray(bo, dtype=np.float32)

    variant, m = _classify_mask(mask)
    nc = _get_prog(variant)

    bf = ml_dtypes.bfloat16
    # xt: [128, NCH, ND, CH] stripe-major partition-major layout of x[b].T
    xt = [np.ascontiguousarray(
        x[b].T.reshape(ND, 128, NCH, CH).transpose(1, 2, 0, 3)).astype(bf)
        for b in range(B)]
    # stripe 0 in s-tile-major layout: [128, 4 s-tiles, ND, 128]
    xt0 = [np.ascontiguousarray(
        x[b].T[:, :CH].reshape(ND, 128, 4, 128).transpose(1, 2, 0, 3))
        .astype(bf) for b in range(B)]
    if variant == "masked":
        mkf = _full_masks(m)

    def _pack_w(w):  # [D, GD] -> [128, ND, GD]
        return np.ascontiguousarray(
            w.reshape(ND, 128, GD).transpose(1, 0, 2)).astype(bf)

    id64 = np.zeros((HD, 128), dtype=np.float32)
    id64[np.arange(HD), HD + np.arange(HD)] = 1.0

    in_maps = []
    for c in range(NCORES):
        b, g = c // (NCORES // B), c % (NCORES // B)
        gs = slice(g * GD, (g + 1) * GD)
        im = {
            "xt": xt[b],
            "xt0": xt0[b],
            "wq4": _pack_w(wq[:, gs]),
            "wk4": _pack_w(wk[:, gs]),
            "wv4": _pack_w(wv[:, gs]),
            "wo4": np.ascontiguousarray(
                wo[gs, :].reshape(2, 128, D).transpose(1, 0, 2)).astype(bf),
            "id64": id64.astype(bf),
            "bq4": np.ascontiguousarray(bq[gs]),
            "bk4": np.ascontiguousarray(bk[gs]),
            "bv4": np.ascontiguousarray(bv[gs]),
            "bo1": bo if g == 0 else np.zeros_like(bo),
        }
        if variant == "causal":
            im["tri"] = _tri_mask()
        elif variant == "masked":
            im["mkf"] = mkf
        in_maps.append(im)

    res = run_bass_kernel_spmd(nc, in_maps, core_ids=list(range(NCORES)))
    out = np.zeros((B, S, D), dtype=np.float32)
    for c in range(NCORES):
        r = res.results[c]["out"]  # [128, NCH, 2, ND//2, CH] bf16
        ft = r.astype(np.float32).transpose(2, 3, 0, 1, 4).reshape(D, S)
        out[c // (NCORES // B)] += ft.T
    return out

